# revision 19
# baseline (speedup 1.0000x reference)
"""AlphaNet feature-extractor kernel for 8 Trainium2 NeuronCores.

Strategy (pure data parallel, batch sharded 8 ways):
  - Host folds every BatchNorm + the avg-pool branch + feature permutation
    into an effective W1e [585, 30] / b1e [30] acting on RAW features.
  - Per core, per super-tile of 512 samples (4 samples per partition):
    centered cross-moments via one rotation-trick tensor_tensor over bf16,
    one fused segmented reduce, rational features (corr/zsc/ret), max/min
    pools over the 3 windows, then a PE-transposed contraction with the
    folded 2-layer MLP.

Kernel raw feature layout F[585], bf16, w-major: F[0:351] = [3w x 117ch],
channel order per w-plane (POOL_BASE): S(0:9), C-pairs(9:45), Dc(45:54),
stdu(54:63), zsc(63:72), corr(72:108), ret(108:117); then MX[117] at
351:468 and MN[117] at 468:585 (max/min over the 3 windows).
Raw-to-reference: mean=S/10, cov=C/10, decay=Dc/55, std_u=stdu/3,
zsc_ref=0.3*zsc, corr same, ret_ref=ret-2 - all folded into W1e/b1e.
"""
import numpy as np

EPS = 1e-5
NF = 585
NCORES = 8
B_TOTAL = 131072
B_CORE = B_TOTAL // NCORES          # 16384
ST_SAMPLES = 512                    # samples per super-tile (4 per partition)
R = 4
N_ST = B_CORE // ST_SAMPLES         # 32
CHUNKS = [(0, 128), (128, 128), (256, 128), (384, 128), (512, 73)]

FE_ALPHA = [1.0, 0.1, 1.0 / 3.0, 0.3, 1.0, 1.0 / 55.0, 0.1]
FE_BETA = [0.0, 0.0, 0.0, 0.0, -2.0, 0.0, 0.0]
FE_BASE = [0, 108, 216, 243, 270, 297, 324]
# feats order: [corr, cov(C), std, zsc, ret, decay(Dc), mean(S)]
POOL_BASE = [72, 9, 54, 63, 108, 45, 0]


def _ref_pair_to_kernel(p_ref, jj, kk):
    a, b = int(jj[p_ref]), int(kk[p_ref])
    g = b - a
    if g <= 4:
        k, j = g, a
    else:
        k, j = 9 - g, b
    return 9 * (k - 1) + j


def fold_params(bn1_h, bn1_n, bn2_h, bn2_n, W1, b1):
    jj, kk = np.triu_indices(9, 1)
    W1 = np.asarray(W1, np.float64)
    W1e = np.zeros((NF, 30), np.float64)
    b1e = np.asarray(b1, np.float64).copy()
    bn1 = [bn1_h[0], bn1_h[1], bn1_n[0], bn1_n[1], bn1_n[2], bn1_n[3], bn1_n[4]]
    sizes = [(36 if i < 2 else 9) for i in range(7) for q in range(3)]
    fp_starts = np.concatenate([[0], np.cumsum(sizes)]) + 351

    def bn2_get(i, q):
        return bn2_h[3 * i + q] if i < 2 else bn2_n[3 * (i - 2) + q]

    for i in range(7):
        Cch = 36 if i < 2 else 9
        a0, b0 = FE_ALPHA[i], FE_BETA[i]
        g1, bb1, rm1, rv1 = [np.asarray(x, np.float64) for x in bn1[i]]
        s1 = g1 / np.sqrt(rv1 + EPS)
        for c in range(Cch):
            kp = _ref_pair_to_kernel(c, jj, kk) if i < 2 else c
            kidx = [117 * w + POOL_BASE[i] + kp for w in range(3)]
            sh = s1[c] * b0 + bb1[c] - s1[c] * rm1[c]
            coef = s1[c] * a0
            for w in range(3):
                ref = FE_BASE[i] + c * 3 + w
                W1e[kidx[w]] += W1[:, ref] * coef
                b1e += W1[:, ref] * sh
            pc = POOL_BASE[i] + kp
            for q in range(3):
                g2, bb2, rm2, rv2 = [np.asarray(x, np.float64) for x in bn2_get(i, q)]
                s2 = g2 / np.sqrt(rv2 + EPS)
                ref = int(fp_starts[3 * i + q]) + c
                if q == 0:
                    for w in range(3):
                        W1e[kidx[w]] += W1[:, ref] * (s2[c] * coef / 3.0)
                    b1e += W1[:, ref] * (s2[c] * (sh - rm2[c]) + bb2[c])
                else:
                    use_max = (q == 1) if coef >= 0 else (q != 1)
                    idx = (351 + pc) if use_max else (468 + pc)
                    W1e[idx] += W1[:, ref] * (s2[c] * coef)
                    b1e += W1[:, ref] * (s2[c] * (sh - rm2[c]) + bb2[c])
    return W1e.astype(np.float32), b1e.astype(np.float32)


_CACHE = {}


def _ap(full, off, dims):
    """Raw AP on the same tensor: keep partition dim of `full`, free dims = dims."""
    import concourse.bass as bass
    return bass.AP(tensor=full.tensor, offset=full.offset + off,
                   ap=[list(full.ap[0])] + [[s, c] for (s, c) in dims])


def build_graph(b_core=B_CORE, reps=1):
    import concourse.bacc as bacc
    import concourse.tile as tile
    from concourse import mybir
    from concourse.mybir import AluOpType as Op

    n_st = b_core // ST_SAMPLES
    f32, bf16 = mybir.dt.float32, mybir.dt.bfloat16
    nc = bacc.Bacc("TRN2", target_bir_lowering=False, debug=False)

    Xd = nc.dram_tensor("X", [b_core, 270], f32, kind="ExternalInput")
    w1d = nc.dram_tensor("w1e", [NF, 30], bf16, kind="ExternalInput")
    b1d = nc.dram_tensor("b1e", [30, 1], f32, kind="ExternalInput")
    w2d = nc.dram_tensor("w2e", [30, 1], bf16, kind="ExternalInput")
    wtd = nc.dram_tensor("wts", [128, 30], bf16, kind="ExternalInput")
    idd = nc.dram_tensor("ident", [128, 128], bf16, kind="ExternalInput")
    outd = nc.dram_tensor("out", [b_core, 1], f32, kind="ExternalOutput")

    Xv = Xd.ap().rearrange("(s q) t -> s q t", q=ST_SAMPLES)   # [n_st, 512, 270]
    outv = outd.ap().rearrange("a b -> b a")                    # [1, b_core]

    with tile.TileContext(nc) as tc:
        with (
            tc.tile_pool(name="consts", bufs=1) as consts,
            tc.tile_pool(name="work", bufs=3) as work,
            tc.tile_pool(name="feat", bufs=2) as feat,
            tc.tile_pool(name="small", bufs=3) as small,
            tc.tile_pool(name="ft", bufs=2) as ftp,
            tc.tile_pool(name="stage", bufs=3) as stagep,
            tc.tile_pool(name="psum_t", bufs=3, space="PSUM") as pst,
            tc.tile_pool(name="psum_h", bufs=2, space="PSUM") as psh,
            tc.tile_pool(name="psum_o", bufs=2, space="PSUM") as pso,
        ):
            w1sb = []
            for (c0, cl) in CHUNKS:
                t = consts.tile([cl, 30], bf16, tag=f"w1_{c0}")
                nc.sync.dma_start(out=t, in_=w1d.ap()[c0:c0 + cl, :])
                w1sb.append(t)
            b1sb = consts.tile([30, 1], f32, tag="b1")
            nc.sync.dma_start(out=b1sb, in_=b1d.ap())
            w2sb = consts.tile([30, 1], bf16, tag="w2")
            nc.sync.dma_start(out=w2sb, in_=w2d.ap())
            wtsb = consts.tile([128, 30], bf16, tag="wts")
            nc.sync.dma_start(out=wtsb, in_=wtd.ap())
            idsb = consts.tile([128, 128], bf16, tag="ident")
            nc.sync.dma_start(out=idsb, in_=idd.ap())

            for st in [st_ for _ in range(reps) for st_ in range(n_st)]:
                X = work.tile([128, 4, 270], f32, tag="X")
                nc.sync.dma_start(
                    out=X[:, :, :].rearrange("p r t -> p (r t)"),
                    in_=Xv[st].rearrange("(p r) t -> p (r t)", r=R))
                Xf = X[:, :, :]

                F = feat.tile([128, 4, NF], bf16, tag="F")
                Ff = F[:, :, :]
                G = small.tile([128, 4, 189], f32, tag="G")
                Gf = G[:, :, :]
                meanX = work.tile([128, 4, 270], bf16, tag="meanX")
                xm2 = work.tile([128, 4, 540], bf16, tag="xm2")
                xm2f = xm2[:, :, :]
                Rin = work.tile([128, 4, 1620], bf16, tag="Rin")   # [P(1080)|Dwt(270)|sq(270)]
                Rinf = Rin[:, :, :]
                halfP = work.tile([128, 4, 810], bf16, tag="halfP")
                hPf = halfP[:, :, :]
                Xb = work.tile([128, 4, 270], bf16, tag="Xb")
                meanT = small.tile([128, 4, 27], f32, tag="mean")
                rx0 = small.tile([128, 4, 27], f32, tag="rx0")
                Q = small.tile([128, 4, 135], f32, tag="Q")       # [rV|rmul]
                Qs = small.tile([128, 4, 135], f32, tag="Qs")     # [srV|rsd]
                rVd = small.tile([128, 4, 54], f32, tag="rVd")
                Qf = Q[:, :, :]

                # S = sum_d X  -> G[:,:,0:27]
                nc.vector.reduce_sum(
                    out=_ap(Gf, 0, [(189, 4), (1, 27)]),
                    in_=_ap(Xf, 0, [(10, 108), (1, 10)]),
                    axis=mybir.AxisListType.X)
                # mean = 0.1 * S (ACT)
                nc.scalar.mul(out=meanT[:, :, :], in_=G[:, :, 0:27], mul=0.1)
                # bf16 shadow of X and broadcast mean (ACT) for 2x-mode xm
                nc.scalar.copy(out=Xb[:, :, :], in_=Xf)
                nc.scalar.copy(
                    out=meanX[:, :, :],
                    in_=_ap(meanT[:, :, :], 0, [(27, 4), (1, 27), (0, 10)]))
                # xm = Xb - mean (bf16, 2x mode)
                nc.vector.tensor_tensor(
                    out=_ap(xm2f, 0, [(540, 4), (1, 270)]),
                    in0=_ap(Xb[:, :, :], 0, [(270, 4), (1, 270)]),
                    in1=_ap(meanX[:, :, :], 0, [(270, 4), (1, 270)]),
                    op=Op.subtract)
                # duplicate xm for channel rotation (ACT)
                nc.scalar.copy(
                    out=_ap(xm2f, 270, [(540, 4), (1, 270)]),
                    in_=_ap(xm2f, 0, [(540, 4), (1, 270)]))
                # decay weighted product on raw X: Dwt = Xb * wts
                nc.vector.tensor_tensor(
                    out=_ap(Rinf, 1080, [(1620, 4), (1, 270)]),
                    in0=_ap(Xb[:, :, :], 0, [(270, 4), (1, 270)]),
                    in1=_ap(wtsb[:, :], 0, [(0, 4), (0, 9), (1, 30)]),
                    op=Op.mult)
                # products: [xm*rot1 .. xm*rot4 | xm^2]
                nc.vector.tensor_tensor(
                    out=_ap(Rinf, 0, [(1620, 4), (1, 1080)]),
                    in0=_ap(xm2f, 0, [(540, 4), (0, 4), (1, 270)]),
                    in1=_ap(xm2f, 30, [(540, 4), (30, 4), (1, 270)]),
                    op=Op.mult)
                nc.scalar.activation(
                    out=_ap(Rinf, 1350, [(1620, 4), (1, 270)]),
                    in_=_ap(xm2f, 0, [(540, 4), (1, 270)]),
                    func=mybir.ActivationFunctionType.Square)
                # 2-stage segmented reduce: halve d via bf16 TT add (2x mode),
                # then reduce-5 -> G[:,:,27:189] = [C(108) | Dc(27) | V(27)]
                nc.vector.tensor_tensor(
                    out=_ap(hPf, 0, [(810, 4), (1, 810)]),
                    in0=_ap(Rinf, 0, [(1620, 4), (10, 162), (1, 5)]),
                    in1=_ap(Rinf, 5, [(1620, 4), (10, 162), (1, 5)]),
                    op=Op.add)
                nc.vector.reduce_sum(
                    out=_ap(Gf, 27, [(189, 4), (1, 162)]),
                    in_=_ap(hPf, 0, [(5, 648), (1, 5)]),
                    axis=mybir.AxisListType.X)
                # rV = 1/V
                nc.vector.reciprocal_approx_fast(
                    out=Q[:, :, 0:27], in_=G[:, :, 162:189])
                # rmul[k,n,w] = rV[n,w] * rV[n+k,w] via doubled rV
                nc.scalar.copy(
                    out=rVd[:, :, :],
                    in_=_ap(Qf, 0, [(135, 4), (0, 2), (1, 27)]))
                nc.vector.tensor_tensor(
                    out=_ap(Qf, 27, [(135, 4), (1, 108)]),
                    in0=_ap(rVd[:, :, :], 0, [(54, 4), (0, 4), (1, 27)]),
                    in1=_ap(rVd[:, :, :], 3, [(54, 4), (3, 4), (1, 27)]),
                    op=Op.mult)
                # sqrt of [rV | rmul] -> [srV | rsd]
                nc.scalar.sqrt(out=Qs[:, :, :], in_=Q[:, :, :])
                # stdu = sqrt(V) -> Fb ch 54..62
                nc.scalar.sqrt(
                    out=_ap(Ff, 54, [(585, 4), (1, 9), (117, 3)]),
                    in_=G[:, :, 162:189])
                # cast S+C+Dc -> Fb ch 0..53
                nc.scalar.copy(
                    out=_ap(Ff, 0, [(585, 4), (1, 54), (117, 3)]),
                    in_=_ap(Gf, 0, [(189, 4), (3, 54), (1, 3)]))
                # merged [zsc | corr] = [S | C] * [srV | rsd] -> Fb ch 63..107
                nc.vector.tensor_tensor(
                    out=_ap(Ff, 63, [(585, 4), (1, 45), (117, 3)]),
                    in0=G[:, :, 0:135],
                    in1=Qs[:, :, :], op=Op.mult)
                # ret = X9/X0
                nc.vector.reciprocal_approx_fast(
                    out=rx0[:, :, :],
                    in_=_ap(Xf, 0, [(270, 4), (10, 27)]))
                nc.vector.tensor_tensor(
                    out=_ap(Ff, 108, [(585, 4), (1, 9), (117, 3)]),
                    in0=_ap(Xf, 9, [(270, 4), (10, 27)]),
                    in1=rx0[:, :, :], op=Op.mult)
                # pools: max/min over w via TT chains
                ptmp = small.tile([128, 4, 117], bf16, tag="ptmp")
                ptm2 = small.tile([128, 4, 117], bf16, tag="ptm2")
                nc.vector.tensor_tensor(
                    out=ptmp[:, :, :],
                    in0=_ap(Ff, 0, [(585, 4), (1, 117)]),
                    in1=_ap(Ff, 117, [(585, 4), (1, 117)]), op=Op.max)
                nc.vector.tensor_tensor(
                    out=_ap(Ff, 351, [(585, 4), (1, 117)]),
                    in0=ptmp[:, :, :],
                    in1=_ap(Ff, 234, [(585, 4), (1, 117)]), op=Op.max)
                nc.vector.tensor_tensor(
                    out=ptm2[:, :, :],
                    in0=_ap(Ff, 0, [(585, 4), (1, 117)]),
                    in1=_ap(Ff, 117, [(585, 4), (1, 117)]), op=Op.min)
                nc.vector.tensor_tensor(
                    out=_ap(Ff, 468, [(585, 4), (1, 117)]),
                    in0=ptm2[:, :, :],
                    in1=_ap(Ff, 234, [(585, 4), (1, 117)]), op=Op.min)

                # ---- MLP ----
                h1p = psh.tile([30, 512], f32, tag="h1")
                fts = []
                for ci, (c0, cl) in enumerate(CHUNKS):
                    ft = ftp.tile([128, 512], bf16, tag=f"ft{ci}")
                    fts.append(ft)
                for ci, (c0, cl) in enumerate(CHUNKS):
                    tp = pst.tile([128, 512], bf16, tag="tp")
                    for r in range(R):
                        nc.tensor.transpose(
                            tp[:cl, r * 128:(r + 1) * 128],
                            F[:, r, c0:c0 + cl], idsb[:, :])
                    nc.scalar.copy(out=fts[ci][:cl, :], in_=tp[:cl, :])
                for ci, (c0, cl) in enumerate(CHUNKS):
                    nc.tensor.matmul(
                        h1p[:, :], w1sb[ci][:, :], fts[ci][:cl, :],
                        start=(ci == 0), stop=(ci == len(CHUNKS) - 1))
                h1s = small.tile([30, 512], bf16, tag="h1s")
                nc.scalar.activation(
                    out=h1s[:, :], in_=h1p[:, :],
                    func=mybir.ActivationFunctionType.Relu,
                    bias=b1sb[:, :], scale=1.0)
                o2 = pso.tile([1, 512], f32, tag="o2")
                nc.tensor.matmul(o2[:, :], w2sb[:, :], h1s[:, :],
                                 start=True, stop=True)
                stg = stagep.tile([1, 512], f32, tag="stg")
                nc.scalar.copy(out=stg[:, :], in_=o2[:, :])
                nc.sync.dma_start(out=outv[:, st * 512:(st + 1) * 512],
                                  in_=stg[:, :])
    nc.compile()
    return nc


def _get_graph(b_core):
    key = b_core
    if key not in _CACHE:
        _CACHE[key] = build_graph(b_core)
    return _CACHE[key]


def kernel(X, bn1_h, bn1_n, bn2_h, bn2_n, W1, b1, W2, b2):
    import ml_dtypes
    from concourse.bass_utils import run_bass_kernel_spmd

    X = np.ascontiguousarray(np.asarray(X, np.float32).reshape(B_TOTAL, 270))
    W1e, b1e = fold_params(np.asarray(bn1_h), np.asarray(bn1_n),
                           np.asarray(bn2_h), np.asarray(bn2_n),
                           np.asarray(W1), np.asarray(b1))
    w1e_bf = W1e.astype(ml_dtypes.bfloat16)
    w2e_bf = np.asarray(W2, np.float32).reshape(30, 1).astype(ml_dtypes.bfloat16)
    wts = np.tile(np.concatenate([np.arange(1, 11, dtype=np.float32)] * 3).reshape(1, 30),
                  (128, 1)).astype(ml_dtypes.bfloat16)
    ident = np.eye(128).astype(ml_dtypes.bfloat16)
    b1e2 = b1e.reshape(30, 1)

    nc = _get_graph(B_CORE)
    in_maps = []
    for c in range(NCORES):
        in_maps.append({
            "X": X[c * B_CORE:(c + 1) * B_CORE],
            "w1e": w1e_bf, "b1e": b1e2, "w2e": w2e_bf,
            "wts": wts, "ident": ident,
        })
    res = run_bass_kernel_spmd(nc, in_maps, core_ids=list(range(NCORES)))
    outs = []
    for c in range(NCORES):
        o = res.results[c]["out"].reshape(N_ST, 4, 128)
        outs.append(o.transpose(0, 2, 1).reshape(B_CORE, 1))
    out = np.concatenate(outs, axis=0)
    return (out + np.asarray(b2, np.float32).reshape(1, 1)).astype(np.float32)


# revision 20
# speedup vs baseline: 1.4755x; 1.4755x over previous
"""AlphaNet feature-extractor kernel for 8 Trainium2 NeuronCores.

Strategy (pure data parallel, batch sharded 8 ways):
  - Host folds every BatchNorm + the avg-pool branch + feature permutation
    into an effective W1e [585, 30] / b1e [30] acting on RAW features.
  - Per core, per super-tile of 512 samples (4 samples per partition):
    centered cross-moments via one rotation-trick tensor_tensor over bf16,
    one fused segmented reduce, rational features (corr/zsc/ret), max/min
    pools over the 3 windows, then a PE-transposed contraction with the
    folded 2-layer MLP.

Kernel raw feature layout F[585], bf16, w-major: F[0:351] = [3w x 117ch],
channel order per w-plane (POOL_BASE): S(0:9), C-pairs(9:45), Dc(45:54),
stdu(54:63), zsc(63:72), corr(72:108), ret(108:117); then MX[117] at
351:468 and MN[117] at 468:585 (max/min over the 3 windows).
Raw-to-reference: mean=S/10, cov=C/10, decay=Dc/55, std_u=stdu/3,
zsc_ref=0.3*zsc, corr same, ret_ref=ret-2 - all folded into W1e/b1e.
"""
import numpy as np

EPS = 1e-5
NF = 585
NCORES = 8
B_TOTAL = 131072
B_CORE = B_TOTAL // NCORES          # 16384
ST_SAMPLES = 512                    # samples per super-tile (4 per partition)
R = 4
N_ST = B_CORE // ST_SAMPLES         # 32
CHUNKS = [(0, 128), (128, 128), (256, 128), (384, 128), (512, 73)]

FE_ALPHA = [1.0, 0.1, 1.0 / 3.0, 0.3, 1.0, 1.0 / 55.0, 0.1]
FE_BETA = [0.0, 0.0, 0.0, 0.0, -2.0, 0.0, 0.0]
FE_BASE = [0, 108, 216, 243, 270, 297, 324]
# feats order: [corr, cov(C), std, zsc, ret, decay(Dc), mean(S)]
POOL_BASE = [72, 9, 54, 63, 108, 45, 0]


def _ref_pair_to_kernel(p_ref, jj, kk):
    a, b = int(jj[p_ref]), int(kk[p_ref])
    g = b - a
    if g <= 4:
        k, j = g, a
    else:
        k, j = 9 - g, b
    return 9 * (k - 1) + j


def fold_params(bn1_h, bn1_n, bn2_h, bn2_n, W1, b1):
    jj, kk = np.triu_indices(9, 1)
    W1 = np.asarray(W1, np.float64)
    W1e = np.zeros((NF, 30), np.float64)
    b1e = np.asarray(b1, np.float64).copy()
    bn1 = [bn1_h[0], bn1_h[1], bn1_n[0], bn1_n[1], bn1_n[2], bn1_n[3], bn1_n[4]]
    sizes = [(36 if i < 2 else 9) for i in range(7) for q in range(3)]
    fp_starts = np.concatenate([[0], np.cumsum(sizes)]) + 351

    def bn2_get(i, q):
        return bn2_h[3 * i + q] if i < 2 else bn2_n[3 * (i - 2) + q]

    for i in range(7):
        Cch = 36 if i < 2 else 9
        a0, b0 = FE_ALPHA[i], FE_BETA[i]
        g1, bb1, rm1, rv1 = [np.asarray(x, np.float64) for x in bn1[i]]
        s1 = g1 / np.sqrt(rv1 + EPS)
        for c in range(Cch):
            kp = _ref_pair_to_kernel(c, jj, kk) if i < 2 else c
            kidx = [117 * w + POOL_BASE[i] + kp for w in range(3)]
            sh = s1[c] * b0 + bb1[c] - s1[c] * rm1[c]
            coef = s1[c] * a0
            for w in range(3):
                ref = FE_BASE[i] + c * 3 + w
                W1e[kidx[w]] += W1[:, ref] * coef
                b1e += W1[:, ref] * sh
            pc = POOL_BASE[i] + kp
            for q in range(3):
                g2, bb2, rm2, rv2 = [np.asarray(x, np.float64) for x in bn2_get(i, q)]
                s2 = g2 / np.sqrt(rv2 + EPS)
                ref = int(fp_starts[3 * i + q]) + c
                if q == 0:
                    for w in range(3):
                        W1e[kidx[w]] += W1[:, ref] * (s2[c] * coef / 3.0)
                    b1e += W1[:, ref] * (s2[c] * (sh - rm2[c]) + bb2[c])
                else:
                    use_max = (q == 1) if coef >= 0 else (q != 1)
                    idx = (351 + pc) if use_max else (468 + pc)
                    W1e[idx] += W1[:, ref] * (s2[c] * coef)
                    b1e += W1[:, ref] * (s2[c] * (sh - rm2[c]) + bb2[c])
    return W1e.astype(np.float32), b1e.astype(np.float32)


_CACHE = {}


def _ap(full, off, dims):
    """Raw AP on the same tensor: keep partition dim of `full`, free dims = dims."""
    import concourse.bass as bass
    return bass.AP(tensor=full.tensor, offset=full.offset + off,
                   ap=[list(full.ap[0])] + [[s, c] for (s, c) in dims])


def build_graph(b_core=B_CORE, reps=1):
    import concourse.bacc as bacc
    import concourse.tile as tile
    from concourse import mybir
    from concourse.mybir import AluOpType as Op

    n_st = b_core // ST_SAMPLES
    f32, bf16 = mybir.dt.float32, mybir.dt.bfloat16
    nc = bacc.Bacc("TRN2", target_bir_lowering=False, debug=False)

    Xd = nc.dram_tensor("X", [b_core, 270], f32, kind="ExternalInput")
    w1d = nc.dram_tensor("w1e", [NF, 30], bf16, kind="ExternalInput")
    b1d = nc.dram_tensor("b1e", [30, 1], f32, kind="ExternalInput")
    w2d = nc.dram_tensor("w2e", [30, 1], bf16, kind="ExternalInput")
    wtd = nc.dram_tensor("wts", [128, 30], bf16, kind="ExternalInput")
    idd = nc.dram_tensor("ident", [128, 128], bf16, kind="ExternalInput")
    outd = nc.dram_tensor("out", [b_core, 1], f32, kind="ExternalOutput")

    Xv = Xd.ap().rearrange("(s q) t -> s q t", q=ST_SAMPLES)   # [n_st, 512, 270]
    outv = outd.ap().rearrange("a b -> b a")                    # [1, b_core]

    with tile.TileContext(nc) as tc:
        with (
            tc.tile_pool(name="consts", bufs=1) as consts,
            tc.tile_pool(name="work", bufs=3) as work,
            tc.tile_pool(name="feat", bufs=2) as feat,
            tc.tile_pool(name="small", bufs=3) as small,
            tc.tile_pool(name="ft", bufs=2) as ftp,
            tc.tile_pool(name="stage", bufs=3) as stagep,
            tc.tile_pool(name="psum_t", bufs=3, space="PSUM") as pst,
            tc.tile_pool(name="psum_h", bufs=2, space="PSUM") as psh,
            tc.tile_pool(name="psum_o", bufs=2, space="PSUM") as pso,
        ):
            w1sb = []
            for (c0, cl) in CHUNKS:
                t = consts.tile([cl, 30], bf16, tag=f"w1_{c0}")
                nc.sync.dma_start(out=t, in_=w1d.ap()[c0:c0 + cl, :])
                w1sb.append(t)
            b1sb = consts.tile([30, 1], f32, tag="b1")
            nc.sync.dma_start(out=b1sb, in_=b1d.ap())
            w2sb = consts.tile([30, 1], bf16, tag="w2")
            nc.sync.dma_start(out=w2sb, in_=w2d.ap())
            wtsb = consts.tile([128, 30], bf16, tag="wts")
            nc.sync.dma_start(out=wtsb, in_=wtd.ap())
            idsb = consts.tile([128, 128], bf16, tag="ident")
            nc.sync.dma_start(out=idsb, in_=idd.ap())

            for st in [st_ for _ in range(reps) for st_ in range(n_st)]:
                X = work.tile([128, 4, 270], f32, tag="X")
                nc.sync.dma_start(
                    out=X[:, :, :].rearrange("p r t -> p (r t)"),
                    in_=Xv[st].rearrange("(p r) t -> p (r t)", r=R))
                Xf = X[:, :, :]

                F = feat.tile([128, 4, NF], bf16, tag="F")
                Ff = F[:, :, :]
                G = small.tile([128, 4, 189], f32, tag="G")
                Gf = G[:, :, :]
                meanX = work.tile([128, 4, 270], bf16, tag="meanX")
                xm2 = work.tile([128, 4, 540], bf16, tag="xm2")
                xm2f = xm2[:, :, :]
                Rin = work.tile([128, 4, 1620], bf16, tag="Rin")   # [P(1080)|Dwt(270)|sq(270)]
                Rinf = Rin[:, :, :]
                halfP = work.tile([128, 4, 810], bf16, tag="halfP")
                hPf = halfP[:, :, :]
                Xb = work.tile([128, 4, 270], bf16, tag="Xb")
                meanT = small.tile([128, 4, 27], f32, tag="mean")
                rx0 = small.tile([128, 4, 27], f32, tag="rx0")
                Q = small.tile([128, 4, 135], f32, tag="Q")       # [rV|rmul]
                Qs = small.tile([128, 4, 135], f32, tag="Qs")     # [srV|rsd]
                rVd = small.tile([128, 4, 54], f32, tag="rVd")
                Qf = Q[:, :, :]

                # S = sum_d X  -> G[:,:,0:27]
                nc.vector.reduce_sum(
                    out=_ap(Gf, 0, [(189, 4), (1, 27)]),
                    in_=_ap(Xf, 0, [(10, 108), (1, 10)]),
                    axis=mybir.AxisListType.X)
                # mean = 0.1 * S (ACT)
                nc.scalar.mul(out=meanT[:, :, :], in_=G[:, :, 0:27], mul=0.1)
                # bf16 shadow of X and broadcast mean (ACT) for 2x-mode xm
                nc.scalar.copy(out=Xb[:, :, :], in_=Xf)
                nc.scalar.copy(
                    out=meanX[:, :, :],
                    in_=_ap(meanT[:, :, :], 0, [(27, 4), (1, 27), (0, 10)]))
                # xm = Xb - mean (bf16, 2x mode)
                nc.vector.tensor_tensor(
                    out=_ap(xm2f, 0, [(540, 4), (1, 270)]),
                    in0=_ap(Xb[:, :, :], 0, [(270, 4), (1, 270)]),
                    in1=_ap(meanX[:, :, :], 0, [(270, 4), (1, 270)]),
                    op=Op.subtract)
                # duplicate xm for channel rotation (ACT)
                nc.scalar.copy(
                    out=_ap(xm2f, 270, [(540, 4), (1, 270)]),
                    in_=_ap(xm2f, 0, [(540, 4), (1, 270)]))
                # decay weighted product on raw X: Dwt = Xb * wts
                nc.vector.tensor_tensor(
                    out=_ap(Rinf, 1080, [(1620, 4), (1, 270)]),
                    in0=_ap(Xb[:, :, :], 0, [(270, 4), (1, 270)]),
                    in1=_ap(wtsb[:, :], 0, [(0, 4), (0, 9), (1, 30)]),
                    op=Op.mult)
                # products: [xm*rot1 .. xm*rot4 | xm^2]
                nc.vector.tensor_tensor(
                    out=_ap(Rinf, 0, [(1620, 4), (1, 1080)]),
                    in0=_ap(xm2f, 0, [(540, 4), (0, 4), (1, 270)]),
                    in1=_ap(xm2f, 30, [(540, 4), (30, 4), (1, 270)]),
                    op=Op.mult)
                nc.scalar.activation(
                    out=_ap(Rinf, 1350, [(1620, 4), (1, 270)]),
                    in_=_ap(xm2f, 0, [(540, 4), (1, 270)]),
                    func=mybir.ActivationFunctionType.Square)
                # 2-stage segmented reduce: halve d via bf16 TT add (2x mode),
                # then reduce-5 -> G[:,:,27:189] = [C(108) | Dc(27) | V(27)]
                nc.vector.tensor_tensor(
                    out=_ap(hPf, 0, [(810, 4), (1, 810)]),
                    in0=_ap(Rinf, 0, [(1620, 4), (10, 162), (1, 5)]),
                    in1=_ap(Rinf, 5, [(1620, 4), (10, 162), (1, 5)]),
                    op=Op.add)
                nc.vector.reduce_sum(
                    out=_ap(Gf, 27, [(189, 4), (1, 162)]),
                    in_=_ap(hPf, 0, [(5, 648), (1, 5)]),
                    axis=mybir.AxisListType.X)
                # rV = 1/V
                nc.vector.reciprocal_approx_fast(
                    out=Q[:, :, 0:27], in_=G[:, :, 162:189])
                # rmul[k,n,w] = rV[n,w] * rV[n+k,w] via doubled rV
                nc.scalar.copy(
                    out=rVd[:, :, :],
                    in_=_ap(Qf, 0, [(135, 4), (0, 2), (1, 27)]))
                nc.vector.tensor_tensor(
                    out=_ap(Qf, 27, [(135, 4), (1, 108)]),
                    in0=_ap(rVd[:, :, :], 0, [(54, 4), (0, 4), (1, 27)]),
                    in1=_ap(rVd[:, :, :], 3, [(54, 4), (3, 4), (1, 27)]),
                    op=Op.mult)
                # sqrt of [rV | rmul] -> [srV | rsd]
                nc.scalar.sqrt(out=Qs[:, :, :], in_=Q[:, :, :])
                # stdu = sqrt(V) -> Fb ch 54..62
                nc.scalar.sqrt(
                    out=_ap(Ff, 54, [(585, 4), (1, 9), (117, 3)]),
                    in_=G[:, :, 162:189])
                # cast S+C+Dc -> Fb ch 0..53
                nc.scalar.copy(
                    out=_ap(Ff, 0, [(585, 4), (1, 54), (117, 3)]),
                    in_=_ap(Gf, 0, [(189, 4), (3, 54), (1, 3)]))
                # merged [zsc | corr] = [S | C] * [srV | rsd] -> Fb ch 63..107
                nc.vector.tensor_tensor(
                    out=_ap(Ff, 63, [(585, 4), (1, 45), (117, 3)]),
                    in0=G[:, :, 0:135],
                    in1=Qs[:, :, :], op=Op.mult)
                # ret = X9/X0
                nc.vector.reciprocal_approx_fast(
                    out=rx0[:, :, :],
                    in_=_ap(Xf, 0, [(270, 4), (10, 27)]))
                nc.vector.tensor_tensor(
                    out=_ap(Ff, 108, [(585, 4), (1, 9), (117, 3)]),
                    in0=_ap(Xf, 9, [(270, 4), (10, 27)]),
                    in1=rx0[:, :, :], op=Op.mult)
                # pools: max/min over w via TT chains
                ptmp = small.tile([128, 4, 117], bf16, tag="ptmp")
                ptm2 = small.tile([128, 4, 117], bf16, tag="ptm2")
                nc.vector.tensor_tensor(
                    out=ptmp[:, :, :],
                    in0=_ap(Ff, 0, [(585, 4), (1, 117)]),
                    in1=_ap(Ff, 117, [(585, 4), (1, 117)]), op=Op.max)
                nc.vector.tensor_tensor(
                    out=_ap(Ff, 351, [(585, 4), (1, 117)]),
                    in0=ptmp[:, :, :],
                    in1=_ap(Ff, 234, [(585, 4), (1, 117)]), op=Op.max)
                nc.vector.tensor_tensor(
                    out=ptm2[:, :, :],
                    in0=_ap(Ff, 0, [(585, 4), (1, 117)]),
                    in1=_ap(Ff, 117, [(585, 4), (1, 117)]), op=Op.min)
                nc.vector.tensor_tensor(
                    out=_ap(Ff, 468, [(585, 4), (1, 117)]),
                    in0=ptm2[:, :, :],
                    in1=_ap(Ff, 234, [(585, 4), (1, 117)]), op=Op.min)

                # ---- MLP ----
                h1p = psh.tile([30, 512], f32, tag="h1")
                # pack chunk pairs into one [128, 1024] bf16 PSUM bank so a
                # single wide ACT copy moves two chunks at once
                pairs = [(0, 1), (2, 3), (4,)]
                fts = {}
                for gi, grp in enumerate(pairs):
                    width = 512 * len(grp)
                    tp = pst.tile([128, 1024], bf16, tag="tp")
                    ftg = ftp.tile([128, 1024], bf16, tag=f"ftg{gi}")
                    for k, ci in enumerate(grp):
                        c0, cl = CHUNKS[ci]
                        for r in range(R):
                            nc.tensor.transpose(
                                tp[:cl, k * 512 + r * 128:k * 512 + (r + 1) * 128],
                                F[:, r, c0:c0 + cl], idsb[:, :])
                        fts[ci] = ftg[0:cl, k * 512:(k + 1) * 512]
                    nc.scalar.copy(out=ftg[:, 0:width], in_=tp[:, 0:width])
                for ci, (c0, cl) in enumerate(CHUNKS):
                    nc.tensor.matmul(
                        h1p[:, :], w1sb[ci][:, :], fts[ci],
                        start=(ci == 0), stop=(ci == len(CHUNKS) - 1))
                h1s = small.tile([30, 512], bf16, tag="h1s")
                nc.scalar.activation(
                    out=h1s[:, :], in_=h1p[:, :],
                    func=mybir.ActivationFunctionType.Relu,
                    bias=b1sb[:, :], scale=1.0)
                o2 = pso.tile([1, 512], f32, tag="o2")
                nc.tensor.matmul(o2[:, :], w2sb[:, :], h1s[:, :],
                                 start=True, stop=True)
                stg = stagep.tile([1, 512], f32, tag="stg")
                nc.scalar.copy(out=stg[:, :], in_=o2[:, :])
                nc.sync.dma_start(out=outv[:, st * 512:(st + 1) * 512],
                                  in_=stg[:, :])
    nc.compile()
    return nc


def _get_graph(b_core):
    key = b_core
    if key not in _CACHE:
        _CACHE[key] = build_graph(b_core)
    return _CACHE[key]


def kernel(X, bn1_h, bn1_n, bn2_h, bn2_n, W1, b1, W2, b2):
    import ml_dtypes
    from concourse.bass_utils import run_bass_kernel_spmd

    X = np.ascontiguousarray(np.asarray(X, np.float32).reshape(B_TOTAL, 270))
    W1e, b1e = fold_params(np.asarray(bn1_h), np.asarray(bn1_n),
                           np.asarray(bn2_h), np.asarray(bn2_n),
                           np.asarray(W1), np.asarray(b1))
    w1e_bf = W1e.astype(ml_dtypes.bfloat16)
    w2e_bf = np.asarray(W2, np.float32).reshape(30, 1).astype(ml_dtypes.bfloat16)
    wts = np.tile(np.concatenate([np.arange(1, 11, dtype=np.float32)] * 3).reshape(1, 30),
                  (128, 1)).astype(ml_dtypes.bfloat16)
    ident = np.eye(128).astype(ml_dtypes.bfloat16)
    b1e2 = b1e.reshape(30, 1)

    nc = _get_graph(B_CORE)
    in_maps = []
    for c in range(NCORES):
        in_maps.append({
            "X": X[c * B_CORE:(c + 1) * B_CORE],
            "w1e": w1e_bf, "b1e": b1e2, "w2e": w2e_bf,
            "wts": wts, "ident": ident,
        })
    res = run_bass_kernel_spmd(nc, in_maps, core_ids=list(range(NCORES)))
    outs = []
    for c in range(NCORES):
        o = res.results[c]["out"].reshape(N_ST, 4, 128)
        outs.append(o.transpose(0, 2, 1).reshape(B_CORE, 1))
    out = np.concatenate(outs, axis=0)
    return (out + np.asarray(b2, np.float32).reshape(1, 1)).astype(np.float32)


# revision 24
# speedup vs baseline: 2.7176x; 1.8418x over previous
"""AlphaNet feature-extractor kernel for 8 Trainium2 NeuronCores.

Strategy (pure data parallel, batch sharded 8 ways):
  - Host folds every BatchNorm + the avg-pool branch + feature permutation
    into an effective W1e [585, 30] / b1e [30] acting on RAW features.
  - Per core, per super-tile of 512 samples (4 samples per partition):
    centered cross-moments via one rotation-trick tensor_tensor over bf16,
    one fused segmented reduce, rational features (corr/zsc/ret), max/min
    pools over the 3 windows, then a PE-transposed contraction with the
    folded 2-layer MLP.

Kernel raw feature layout F[585], bf16, w-major: F[0:351] = [3w x 117ch],
channel order per w-plane (POOL_BASE): S(0:9), C-pairs(9:45), Dc(45:54),
stdu(54:63), zsc(63:72), corr(72:108), ret(108:117); then MX[117] at
351:468 and MN[117] at 468:585 (max/min over the 3 windows).
Raw-to-reference: mean=S/10, cov=C/10, decay=Dc/55, std_u=stdu/3,
zsc_ref=0.3*zsc, corr same, ret_ref=ret-2 - all folded into W1e/b1e.
"""
import numpy as np

EPS = 1e-5
NF = 585
NCORES = 8
B_TOTAL = 131072
B_CORE = B_TOTAL // NCORES          # 16384
ST_SAMPLES = 512                    # samples per super-tile (4 per partition)
R = 4
N_ST = B_CORE // ST_SAMPLES         # 32
CHUNKS = [(0, 128), (128, 128), (256, 128), (384, 128), (512, 73)]

FE_ALPHA = [1.0, 0.1, 1.0 / 3.0, 0.3, 1.0, 1.0 / 55.0, 0.1]
FE_BETA = [0.0, 0.0, 0.0, 0.0, -2.0, 0.0, 0.0]
FE_BASE = [0, 108, 216, 243, 270, 297, 324]
# feats order: [corr, cov(C), std, zsc, ret, decay(Dc), mean(S)]
POOL_BASE = [72, 9, 54, 63, 108, 45, 0]


def _ref_pair_to_kernel(p_ref, jj, kk):
    a, b = int(jj[p_ref]), int(kk[p_ref])
    g = b - a
    if g <= 4:
        k, j = g, a
    else:
        k, j = 9 - g, b
    return 9 * (k - 1) + j


def fold_params(bn1_h, bn1_n, bn2_h, bn2_n, W1, b1):
    jj, kk = np.triu_indices(9, 1)
    W1 = np.asarray(W1, np.float64)
    W1e = np.zeros((NF, 30), np.float64)
    b1e = np.asarray(b1, np.float64).copy()
    bn1 = [bn1_h[0], bn1_h[1], bn1_n[0], bn1_n[1], bn1_n[2], bn1_n[3], bn1_n[4]]
    sizes = [(36 if i < 2 else 9) for i in range(7) for q in range(3)]
    fp_starts = np.concatenate([[0], np.cumsum(sizes)]) + 351

    def bn2_get(i, q):
        return bn2_h[3 * i + q] if i < 2 else bn2_n[3 * (i - 2) + q]

    for i in range(7):
        Cch = 36 if i < 2 else 9
        a0, b0 = FE_ALPHA[i], FE_BETA[i]
        g1, bb1, rm1, rv1 = [np.asarray(x, np.float64) for x in bn1[i]]
        s1 = g1 / np.sqrt(rv1 + EPS)
        for c in range(Cch):
            kp = _ref_pair_to_kernel(c, jj, kk) if i < 2 else c
            kidx = [117 * w + POOL_BASE[i] + kp for w in range(3)]
            sh = s1[c] * b0 + bb1[c] - s1[c] * rm1[c]
            coef = s1[c] * a0
            for w in range(3):
                ref = FE_BASE[i] + c * 3 + w
                W1e[kidx[w]] += W1[:, ref] * coef
                b1e += W1[:, ref] * sh
            pc = POOL_BASE[i] + kp
            for q in range(3):
                g2, bb2, rm2, rv2 = [np.asarray(x, np.float64) for x in bn2_get(i, q)]
                s2 = g2 / np.sqrt(rv2 + EPS)
                ref = int(fp_starts[3 * i + q]) + c
                if q == 0:
                    for w in range(3):
                        W1e[kidx[w]] += W1[:, ref] * (s2[c] * coef / 3.0)
                    b1e += W1[:, ref] * (s2[c] * (sh - rm2[c]) + bb2[c])
                else:
                    use_max = (q == 1) if coef >= 0 else (q != 1)
                    idx = (351 + pc) if use_max else (468 + pc)
                    W1e[idx] += W1[:, ref] * (s2[c] * coef)
                    b1e += W1[:, ref] * (s2[c] * (sh - rm2[c]) + bb2[c])
    return W1e.astype(np.float32), b1e.astype(np.float32)


_CACHE = {}


def _ap(full, off, dims):
    """Raw AP on the same tensor: keep partition dim of `full`, free dims = dims."""
    import concourse.bass as bass
    return bass.AP(tensor=full.tensor, offset=full.offset + off,
                   ap=[list(full.ap[0])] + [[s, c] for (s, c) in dims])


def build_graph(b_core=B_CORE, reps=1):
    import concourse.bacc as bacc
    import concourse.tile as tile
    from concourse import mybir
    from concourse.mybir import AluOpType as Op

    n_st = b_core // ST_SAMPLES
    f32, bf16 = mybir.dt.float32, mybir.dt.bfloat16
    nc = bacc.Bacc("TRN2", target_bir_lowering=False, debug=False)

    Xd = nc.dram_tensor("X", [b_core, 270], f32, kind="ExternalInput")
    w1d = nc.dram_tensor("w1e", [NF, 30], bf16, kind="ExternalInput")
    b1d = nc.dram_tensor("b1e", [30, 1], f32, kind="ExternalInput")
    w2d = nc.dram_tensor("w2e", [30, 1], bf16, kind="ExternalInput")
    wtd = nc.dram_tensor("wts", [128, 30], bf16, kind="ExternalInput")
    idd = nc.dram_tensor("ident", [128, 128], bf16, kind="ExternalInput")
    outd = nc.dram_tensor("out", [b_core, 1], f32, kind="ExternalOutput")

    Xv = Xd.ap().rearrange("(s q) t -> s q t", q=ST_SAMPLES)   # [n_st, 512, 270]
    outv = outd.ap().rearrange("a b -> b a")                    # [1, b_core]

    with tile.TileContext(nc) as tc:
        with (
            tc.tile_pool(name="consts", bufs=1) as consts,
            tc.tile_pool(name="work", bufs=3) as work,
            tc.tile_pool(name="feat", bufs=2) as feat,
            tc.tile_pool(name="small", bufs=3) as small,
            tc.tile_pool(name="ft", bufs=2) as ftp,
            tc.tile_pool(name="stage", bufs=3) as stagep,
            tc.tile_pool(name="psum_t", bufs=3, space="PSUM") as pst,
            tc.tile_pool(name="psum_h", bufs=2, space="PSUM") as psh,
            tc.tile_pool(name="psum_o", bufs=2, space="PSUM") as pso,
        ):
            w1sb = []
            for (c0, cl) in CHUNKS:
                t = consts.tile([cl, 30], bf16, tag=f"w1_{c0}")
                nc.sync.dma_start(out=t, in_=w1d.ap()[c0:c0 + cl, :])
                w1sb.append(t)
            b1sb = consts.tile([30, 1], f32, tag="b1")
            nc.sync.dma_start(out=b1sb, in_=b1d.ap())
            w2sb = consts.tile([30, 1], bf16, tag="w2")
            nc.sync.dma_start(out=w2sb, in_=w2d.ap())
            wtsb = consts.tile([128, 30], bf16, tag="wts")
            nc.sync.dma_start(out=wtsb, in_=wtd.ap())
            idsb = consts.tile([128, 128], bf16, tag="ident")
            nc.sync.dma_start(out=idsb, in_=idd.ap())

            for st in [st_ for _ in range(reps) for st_ in range(n_st)]:
                X = work.tile([128, 4, 270], f32, tag="X")
                nc.sync.dma_start(
                    out=X[:, :, :].rearrange("p r t -> p (r t)"),
                    in_=Xv[st].rearrange("(p r) t -> p (r t)", r=R))
                Xf = X[:, :, :]

                F = feat.tile([128, 4, NF], bf16, tag="F")
                Ff = F[:, :, :]
                G = small.tile([128, 4, 189], f32, tag="G")
                Gf = G[:, :, :]
                meanX = work.tile([128, 4, 270], bf16, tag="meanX")
                xm2 = work.tile([128, 4, 540], bf16, tag="xm2")
                xm2f = xm2[:, :, :]
                Rin = work.tile([128, 4, 1620], bf16, tag="Rin")   # [P(1080)|Dwt(270)|sq(270)]
                Rinf = Rin[:, :, :]
                halfP = work.tile([128, 4, 810], bf16, tag="halfP")
                hPf = halfP[:, :, :]
                Xb = work.tile([128, 4, 270], bf16, tag="Xb")
                meanT = small.tile([128, 4, 27], f32, tag="mean")
                rx0 = small.tile([128, 4, 27], f32, tag="rx0")
                Q = small.tile([128, 4, 135], f32, tag="Q")       # [rV|rmul]
                Qs = small.tile([128, 4, 135], f32, tag="Qs")     # [srV|rsd]
                rVd = small.tile([128, 4, 54], f32, tag="rVd")
                Qf = Q[:, :, :]

                # S = sum_d X  -> G[:,:,0:27]
                nc.vector.reduce_sum(
                    out=_ap(Gf, 0, [(189, 4), (1, 27)]),
                    in_=_ap(Xf, 0, [(10, 108), (1, 10)]),
                    axis=mybir.AxisListType.X)
                # mean = 0.1 * S (ACT)
                nc.scalar.mul(out=meanT[:, :, :], in_=G[:, :, 0:27], mul=0.1)
                # bf16 shadow of X and broadcast mean (ACT) for 2x-mode xm
                nc.scalar.copy(out=Xb[:, :, :], in_=Xf)
                nc.scalar.copy(
                    out=meanX[:, :, :],
                    in_=_ap(meanT[:, :, :], 0, [(27, 4), (1, 27), (0, 10)]))
                # xm = Xb - mean (bf16, 2x mode)
                nc.vector.tensor_tensor(
                    out=_ap(xm2f, 0, [(540, 4), (1, 270)]),
                    in0=_ap(Xb[:, :, :], 0, [(270, 4), (1, 270)]),
                    in1=_ap(meanX[:, :, :], 0, [(270, 4), (1, 270)]),
                    op=Op.subtract)
                # duplicate xm for channel rotation (ACT)
                nc.scalar.copy(
                    out=_ap(xm2f, 270, [(540, 4), (1, 270)]),
                    in_=_ap(xm2f, 0, [(540, 4), (1, 270)]))
                # decay weighted product on raw X: Dwt = Xb * wts
                nc.vector.tensor_tensor(
                    out=_ap(Rinf, 1080, [(1620, 4), (1, 270)]),
                    in0=_ap(Xb[:, :, :], 0, [(270, 4), (1, 270)]),
                    in1=_ap(wtsb[:, :], 0, [(0, 4), (0, 9), (1, 30)]),
                    op=Op.mult)
                # products: [xm*rot1 .. xm*rot4 | xm^2]
                nc.vector.tensor_tensor(
                    out=_ap(Rinf, 0, [(1620, 4), (1, 1080)]),
                    in0=_ap(xm2f, 0, [(540, 4), (0, 4), (1, 270)]),
                    in1=_ap(xm2f, 30, [(540, 4), (30, 4), (1, 270)]),
                    op=Op.mult)
                nc.scalar.activation(
                    out=_ap(Rinf, 1350, [(1620, 4), (1, 270)]),
                    in_=_ap(xm2f, 0, [(540, 4), (1, 270)]),
                    func=mybir.ActivationFunctionType.Square)
                # 2-stage segmented reduce: halve d via bf16 TT add (2x mode),
                # then reduce-5 -> G[:,:,27:189] = [C(108) | Dc(27) | V(27)]
                nc.vector.tensor_tensor(
                    out=_ap(hPf, 0, [(810, 4), (1, 810)]),
                    in0=_ap(Rinf, 0, [(1620, 4), (10, 162), (1, 5)]),
                    in1=_ap(Rinf, 5, [(1620, 4), (10, 162), (1, 5)]),
                    op=Op.add)
                nc.vector.reduce_sum(
                    out=_ap(Gf, 27, [(189, 4), (1, 162)]),
                    in_=_ap(hPf, 0, [(5, 648), (1, 5)]),
                    axis=mybir.AxisListType.X)
                # rV = 1/V
                nc.vector.reciprocal_approx_fast(
                    out=Q[:, :, 0:27], in_=G[:, :, 162:189])
                # rmul[k,n,w] = rV[n,w] * rV[n+k,w] via doubled rV
                nc.scalar.copy(
                    out=rVd[:, :, :],
                    in_=_ap(Qf, 0, [(135, 4), (0, 2), (1, 27)]))
                nc.vector.tensor_tensor(
                    out=_ap(Qf, 27, [(135, 4), (1, 108)]),
                    in0=_ap(rVd[:, :, :], 0, [(54, 4), (0, 4), (1, 27)]),
                    in1=_ap(rVd[:, :, :], 3, [(54, 4), (3, 4), (1, 27)]),
                    op=Op.mult)
                # sqrt of [rV | rmul] -> [srV | rsd]
                nc.scalar.sqrt(out=Qs[:, :, :], in_=Q[:, :, :])
                # stdu = sqrt(V) -> Fb ch 54..62
                nc.scalar.sqrt(
                    out=_ap(Ff, 54, [(585, 4), (1, 9), (117, 3)]),
                    in_=G[:, :, 162:189])
                # cast S+C+Dc -> Fb ch 0..53
                nc.scalar.copy(
                    out=_ap(Ff, 0, [(585, 4), (1, 54), (117, 3)]),
                    in_=_ap(Gf, 0, [(189, 4), (3, 54), (1, 3)]))
                # merged [zsc | corr] = [S | C] * [srV | rsd] -> Fb ch 63..107
                nc.vector.tensor_tensor(
                    out=_ap(Ff, 63, [(585, 4), (1, 45), (117, 3)]),
                    in0=G[:, :, 0:135],
                    in1=Qs[:, :, :], op=Op.mult)
                # ret = X9/X0
                nc.vector.reciprocal_approx_fast(
                    out=rx0[:, :, :],
                    in_=_ap(Xf, 0, [(270, 4), (10, 27)]))
                nc.vector.tensor_tensor(
                    out=_ap(Ff, 108, [(585, 4), (1, 9), (117, 3)]),
                    in0=_ap(Xf, 9, [(270, 4), (10, 27)]),
                    in1=rx0[:, :, :], op=Op.mult)
                # pools: max/min over w via TT chains
                ptmp = small.tile([128, 4, 117], bf16, tag="ptmp")
                ptm2 = small.tile([128, 4, 117], bf16, tag="ptm2")
                nc.vector.tensor_tensor(
                    out=ptmp[:, :, :],
                    in0=_ap(Ff, 0, [(585, 4), (1, 117)]),
                    in1=_ap(Ff, 117, [(585, 4), (1, 117)]), op=Op.max)
                nc.vector.tensor_tensor(
                    out=_ap(Ff, 351, [(585, 4), (1, 117)]),
                    in0=ptmp[:, :, :],
                    in1=_ap(Ff, 234, [(585, 4), (1, 117)]), op=Op.max)
                nc.vector.tensor_tensor(
                    out=ptm2[:, :, :],
                    in0=_ap(Ff, 0, [(585, 4), (1, 117)]),
                    in1=_ap(Ff, 117, [(585, 4), (1, 117)]), op=Op.min)
                nc.vector.tensor_tensor(
                    out=_ap(Ff, 468, [(585, 4), (1, 117)]),
                    in0=ptm2[:, :, :],
                    in1=_ap(Ff, 234, [(585, 4), (1, 117)]), op=Op.min)

                # ---- MLP ----
                h1p = psh.tile([30, 512], f32, tag="h1")
                # pack chunk pairs into one [128, 1024] bf16 PSUM bank so a
                # single wide ACT copy moves two chunks at once
                pairs = [(0, 1), (2, 3), (4,)]
                fts = {}
                for gi, grp in enumerate(pairs):
                    width = 512 * len(grp)
                    tp = pst.tile([128, 1024], bf16, tag="tp")
                    ftg = ftp.tile([128, 1024], bf16, tag=f"ftg{gi}")
                    for k, ci in enumerate(grp):
                        c0, cl = CHUNKS[ci]
                        for r in range(R):
                            nc.tensor.transpose(
                                tp[:cl, k * 512 + r * 128:k * 512 + (r + 1) * 128],
                                F[:, r, c0:c0 + cl], idsb[:, :])
                        fts[ci] = ftg[0:cl, k * 512:(k + 1) * 512]
                    nc.scalar.copy(out=ftg[:, 0:width], in_=tp[:, 0:width])
                for ci, (c0, cl) in enumerate(CHUNKS):
                    nc.tensor.matmul(
                        h1p[:, :], w1sb[ci][:, :], fts[ci],
                        start=(ci == 0), stop=(ci == len(CHUNKS) - 1))
                h1s = small.tile([30, 512], bf16, tag="h1s")
                nc.scalar.activation(
                    out=h1s[:, :], in_=h1p[:, :],
                    func=mybir.ActivationFunctionType.Relu,
                    bias=b1sb[:, :], scale=1.0)
                o2 = pso.tile([1, 512], f32, tag="o2")
                nc.tensor.matmul(o2[:, :], w2sb[:, :], h1s[:, :],
                                 start=True, stop=True)
                stg = stagep.tile([1, 512], f32, tag="stg")
                nc.scalar.copy(out=stg[:, :], in_=o2[:, :])
                nc.sync.dma_start(out=outv[:, st * 512:(st + 1) * 512],
                                  in_=stg[:, :])
    nc.compile()
    return nc


def _get_graph(b_core):
    key = b_core
    if key not in _CACHE:
        _CACHE[key] = build_graph(b_core)
    return _CACHE[key]


def kernel(X, bn1_h, bn1_n, bn2_h, bn2_n, W1, b1, W2, b2):
    import ml_dtypes
    from concourse.bass_utils import run_bass_kernel_spmd

    X = np.ascontiguousarray(np.asarray(X, np.float32).reshape(B_TOTAL, 270))
    W1e, b1e = fold_params(np.asarray(bn1_h), np.asarray(bn1_n),
                           np.asarray(bn2_h), np.asarray(bn2_n),
                           np.asarray(W1), np.asarray(b1))
    w1e_bf = W1e.astype(ml_dtypes.bfloat16)
    w2e_bf = np.asarray(W2, np.float32).reshape(30, 1).astype(ml_dtypes.bfloat16)
    wts = np.tile(np.concatenate([np.arange(1, 11, dtype=np.float32)] * 3).reshape(1, 30),
                  (128, 1)).astype(ml_dtypes.bfloat16)
    ident = np.eye(128).astype(ml_dtypes.bfloat16)
    b1e2 = b1e.reshape(30, 1)

    nc = _get_graph(B_CORE)
    in_maps = []
    for c in range(NCORES):
        in_maps.append({
            "X": X[c * B_CORE:(c + 1) * B_CORE],
            "w1e": w1e_bf, "b1e": b1e2, "w2e": w2e_bf,
            "wts": wts, "ident": ident,
        })
    res = run_bass_kernel_spmd(nc, in_maps, core_ids=list(range(NCORES)))
    outs = []
    for c in range(NCORES):
        o = res.results[c]["out"].reshape(N_ST, 4, 128)
        outs.append(o.transpose(0, 2, 1).reshape(B_CORE, 1))
    out = np.concatenate(outs, axis=0)
    return (out + np.asarray(b2, np.float32).reshape(1, 1)).astype(np.float32)
